# revision 10
# baseline (speedup 1.0000x reference)
"""Trainium2 Bass kernel for nn_AdaptiveDecoder (shared MLP + hard-routed type heads).

Strategy:
  * Host: sort nodes by type; each core gets the same static column layout:
    [t0 x 4096 | t1 x 4096 | t2 x 4096 | t0_rem x 128 | t1_rem x 128 | t2_rem x 128]
    so the compiled SPMD program bakes the tile->head mapping in and the
    device does zero routing work.  The three remainder tiles form one final
    384-column block whose head stage switches weights per 128-column segment,
    so the pipeline drains on a single small block.
  * Device: activations stay transposed ([feature, nodes]) so the three matmul
    stages chain without transposes.
  * w2/b2 are mean-centered on the host (per input row, subtract the output-dim
    mean) so stage-2 output is exactly zero-mean: LayerNorm's mean path
    vanishes and variance is just sum(h^2)/H.  The variance column-sum uses an
    all-ones [128,128] lhsT so 1/sigma lands replicated on all partitions --
    no broadcast matmul needed.
  * Stage-1 relu runs on ACT (DVE would gate psum-buffer reuse); DVE keeps the
    squares/pairwise-add/reciprocal chain and the final rsig multiply.
  * All weights packed on the host to the device SBUF layout; first-use
    ordered quarter-sized DMAs keep the startup critical path short.
"""

import sys

sys.path.insert(0, "/opt/trn_rl_repo")

from contextlib import ExitStack

import numpy as np

N_CORES = 8
LATENT, HIDDEN, OUT, TYPES = 512, 1024, 256, 3
P = 128
NB = 512  # node columns per block (psum bank limit for f32)
KL = LATENT // P  # 4 k-tiles, stage 1
KH = HIDDEN // P  # 8 k-tiles, stage 2 / head
MH = HIDDEN // P  # 8 m-chunks of hidden
MO = OUT // P  # 2 m-chunks of head output
Q1 = 256  # w1/w2 quarter width (2 m-chunks)
LN_EPS = 1e-5
FULL = (NB // P) * MH  # 32 full 128-tiles per type region (4096 cols)


def build_program(blocks, R, use_c1=False):
    """blocks: list of (types, col_offset, n_cols) where types is a list of
    (type_idx, seg_cols) head segments covering n_cols."""
    import concourse.mybir as mybir
    import concourse.tile as tile
    from concourse import bacc

    dt = mybir.dt
    f32, bf16 = dt.float32, dt.bfloat16
    AF = mybir.ActivationFunctionType

    nc = bacc.Bacc("TRN2", target_bir_lowering=False, debug=False, num_devices=N_CORES)

    xtd = nc.dram_tensor("xt", [P, KL, R], bf16, kind="ExternalInput").ap()
    w1d = nc.dram_tensor("w1", [P, MH // 2, KL * Q1], bf16, kind="ExternalInput").ap()
    w2d = nc.dram_tensor("w2", [P, MH // 2, KH * Q1], bf16, kind="ExternalInput").ap()
    whpd = nc.dram_tensor("whp", [P, TYPES, KH * OUT], bf16, kind="ExternalInput").ap()
    b1d = nc.dram_tensor("b1r", [P, MH], f32, kind="ExternalInput").ap()
    b2d = nc.dram_tensor("b2r", [P, MH], f32, kind="ExternalInput").ap()
    if use_c1:
        c1d = nc.dram_tensor("c1r", [P, TYPES * MO], f32, kind="ExternalInput").ap()
    outd = nc.dram_tensor("out", [P, MO, R], bf16, kind="ExternalOutput").ap()

    with tile.TileContext(nc) as tc, ExitStack() as ctx:
        consts = ctx.enter_context(tc.tile_pool(name="consts", bufs=1))
        xt_pool = ctx.enter_context(tc.tile_pool(name="xt", bufs=4))
        h1_pool = ctx.enter_context(tc.tile_pool(name="h1", bufs=2))
        h2_pool = ctx.enter_context(tc.tile_pool(name="h2", bufs=2))
        sq_pool = ctx.enter_context(tc.tile_pool(name="sq", bufs=1))
        qs_pool = ctx.enter_context(tc.tile_pool(name="qs", bufs=2))
        rs_pool = ctx.enter_context(tc.tile_pool(name="rs", bufs=2))
        out_pool = ctx.enter_context(tc.tile_pool(name="outp", bufs=2))
        ps_mlp = ctx.enter_context(tc.tile_pool(name="ps_mlp", bufs=4, space="PSUM"))
        ps_head = ctx.enter_context(tc.tile_pool(name="ps_head", bufs=2, space="PSUM"))
        ps_stat = ctx.enter_context(tc.tile_pool(name="ps_stat", bufs=2, space="PSUM"))

        # steady-state DMAs round-robin sync/gpsimd (ACT stays compute-only)
        dma_engines = [nc.sync, nc.gpsimd]
        dma_rr = [0]

        def dma(out, in_):
            eng = dma_engines[dma_rr[0] % len(dma_engines)]
            dma_rr[0] += 1
            eng.dma_start(out=out, in_=in_)

        xt_tiles = {}

        def load_xt(bi, eng=None, split=False):
            _, c0, nb = blocks[bi]
            xt_t = xt_pool.tile([P, KL, NB], bf16, tag="xt")
            if split:  # two k-halves so the first stage-1 group starts sooner
                for ks in range(2):
                    eng.dma_start(
                        out=xt_t[:, 2 * ks : 2 * ks + 2, :nb],
                        in_=xtd[:, 2 * ks : 2 * ks + 2, c0 : c0 + nb],
                    )
            elif eng is not None:
                eng.dma_start(out=xt_t[:, :, :nb], in_=xtd[:, :, c0 : c0 + nb])
            else:
                dma(xt_t[:, :, :nb], xtd[:, :, c0 : c0 + nb])
            xt_tiles[bi] = xt_t

        # --- prologue: interleave the critical-path weight quarters across
        # all four DMA queues in first-use (deadline) order ---
        w1_sb = consts.tile([P, MH // 2, KL * Q1], bf16)
        w2_sb = consts.tile([P, MH // 2, KH * Q1], bf16)
        b1_sb = consts.tile([P, MH], f32)
        b2_sb = consts.tile([P, MH], f32)
        whp_sb = consts.tile([P, TYPES, KH * OUT], bf16)
        t0_first = blocks[0][0][0][0]
        type_order = [t0_first] + [t for t in range(TYPES) if t != t0_first]

        _, _c0, _nb = blocks[0]
        xt0 = xt_pool.tile([P, KL, NB], bf16, tag="xt")
        xt_tiles[0] = xt0
        nc.sync.dma_start(out=xt0[:, 0:2, :_nb], in_=xtd[:, 0:2, _c0 : _c0 + _nb])
        nc.scalar.dma_start(out=w1_sb[:, 0, :], in_=w1d[:, 0, :])
        nc.gpsimd.dma_start(out=b1_sb[:], in_=b1d[:])
        nc.sync.dma_start(out=xt0[:, 2:4, :_nb], in_=xtd[:, 2:4, _c0 : _c0 + _nb])
        nc.scalar.dma_start(out=w1_sb[:, 1, :], in_=w1d[:, 1, :])
        nc.sync.dma_start(out=w1_sb[:, 2, :], in_=w1d[:, 2, :])
        nc.scalar.dma_start(out=w1_sb[:, 3, :], in_=w1d[:, 3, :])
        nc.gpsimd.dma_start(out=b2_sb[:], in_=b2d[:])
        nc.scalar.dma_start(out=w2_sb[:, 0, :], in_=w2d[:, 0, :])
        nc.sync.dma_start(out=w2_sb[:, 1, :], in_=w2d[:, 1, :])
        nc.gpsimd.dma_start(out=w2_sb[:, 2, :], in_=w2d[:, 2, :])
        nc.gpsimd.dma_start(out=w2_sb[:, 3, :], in_=w2d[:, 3, :])
        nc.scalar.dma_start(
            out=whp_sb[:, type_order[0], :], in_=whpd[:, type_order[0], :]
        )
        for bi in range(1, min(3, len(blocks))):
            load_xt(bi, eng=(nc.sync if bi % 2 else nc.gpsimd))
        for ei, t in enumerate(type_order[1:]):
            (nc.scalar if ei % 2 else nc.gpsimd).dma_start(
                out=whp_sb[:, t, :], in_=whpd[:, t, :]
            )
        if use_c1:
            c1_sb = consts.tile([P, TYPES * MO], f32)
            nc.sync.dma_start(out=c1_sb[:], in_=c1d[:])

        ones128 = consts.tile([P, P], bf16)
        nc.vector.memset(ones128[:], 1.0)
        eps_ap = consts.tile([P, 1], f32)
        nc.vector.memset(eps_ap[:], LN_EPS)
        act_warm = consts.tile([1, 1], f32)
        nc.scalar.activation(act_warm[:], eps_ap[:1, :], AF.Sqrt)

        # --- per-block pipeline; tail (rsig multiply + output DMA) of block
        # b-1 is emitted at the top of block b so its DVE ops never gate the
        # PE and the output DMA issues as early as possible ---

        def emit_tail(segs, c0, nb, ph_list, rsig):
            out_sb = out_pool.tile([P, MO, NB], bf16, tag="out")
            for mc in range(MO):
                nc.vector.tensor_mul(
                    out_sb[:, mc, :nb], ph_list[mc][:, :nb], rsig[:, :nb]
                )
                if use_c1:
                    s0 = 0
                    for t, sw in segs:
                        nc.vector.tensor_scalar(
                            out_sb[:, mc, s0 : s0 + sw],
                            out_sb[:, mc, s0 : s0 + sw],
                            c1_sb[:, t * MO + mc : t * MO + mc + 1],
                            0.0,
                            op0=mybir.AluOpType.add,
                            op1=mybir.AluOpType.bypass,
                        )
                        s0 += sw
                dma(outd[:, mc, c0 : c0 + nb], out_sb[:, mc, :nb])

        pending = []
        for bi, (segs, c0, nb) in enumerate(blocks):
            xt_t = xt_tiles.pop(bi)
            if bi + 3 < len(blocks):
                load_xt(bi + 3)

            if pending:
                pending.pop(0)()

            # stage 1: h1^T = relu(W1^T x + b1)   [HIDDEN, nb]  (relu on ACT)
            h1_t = h1_pool.tile([P, MH * NB], bf16, tag="h1")
            for m in range(MH):
                q, i = divmod(m, 2)
                ps = ps_mlp.tile([P, NB], f32, tag="ps_mlp")
                for k in range(KL):
                    nc.tensor.matmul(
                        ps[:, :nb],
                        lhsT=w1_sb[:, q, k * Q1 + i * P : k * Q1 + (i + 1) * P],
                        rhs=xt_t[:, k, :nb],
                        start=(k == 0),
                        stop=(k == KL - 1),
                    )
                nc.scalar.activation(
                    h1_t[:, m * NB : m * NB + nb],
                    ps[:, :nb],
                    AF.Relu,
                    bias=b1_sb[:, m : m + 1],
                )

            # stage 2: h2^T = W2^T h1 + b2 (zero-mean by construction);
            # squares ride along per chunk for the variance sum
            h2_t = h2_pool.tile([P, MH * NB], bf16, tag="h2")
            sq_t = sq_pool.tile([P, MH * NB], bf16, tag="sq")
            for m in range(MH):
                q, i = divmod(m, 2)
                ps = ps_mlp.tile([P, NB], f32, tag="ps_mlp")
                for k in range(KH):
                    nc.tensor.matmul(
                        ps[:, :nb],
                        lhsT=w2_sb[:, q, k * Q1 + i * P : k * Q1 + (i + 1) * P],
                        rhs=h1_t[:, k * NB : k * NB + nb],
                        start=(k == 0),
                        stop=(k == KH - 1),
                    )
                nc.scalar.activation(
                    h2_t[:, m * NB : m * NB + nb],
                    ps[:, :nb],
                    AF.Identity,
                    bias=b2_sb[:, m : m + 1],
                )
                nc.vector.tensor_mul(
                    sq_t[:, m * NB : m * NB + nb],
                    h2_t[:, m * NB : m * NB + nb],
                    h2_t[:, m * NB : m * NB + nb],
                )

            # head main matmuls: keep the PE hot while the stats chain runs.
            # Mixed blocks switch head weights per 128-column segment.
            ph_list = []
            for mc in range(MO):
                ph = ps_head.tile([P, NB], f32, tag="head")
                s0 = 0
                for t, sw in segs:
                    for k in range(KH):
                        nc.tensor.matmul(
                            ph[:, s0 : s0 + sw],
                            lhsT=whp_sb[
                                :, t, k * OUT + mc * P : k * OUT + (mc + 1) * P
                            ],
                            rhs=h2_t[:, k * NB + s0 : k * NB + s0 + sw],
                            start=(k == 0),
                            stop=(k == KH - 1),
                        )
                    s0 += sw
                ph_list.append(ph)

            # variance: pairwise-add squares 8->4->2->1 on DVE, column-sum via
            # ones-matmul (result replicated on all 128 partitions), then
            # sigma = sqrt(sum/H + eps) on ACT and 1/sigma on DVE
            qs_t = qs_pool.tile([P, (MH // 2) * NB], bf16, tag="qs")
            for k in range(MH // 2):
                nc.vector.tensor_add(
                    qs_t[:, k * NB : k * NB + nb],
                    sq_t[:, 2 * k * NB : 2 * k * NB + nb],
                    sq_t[:, (2 * k + 1) * NB : (2 * k + 1) * NB + nb],
                )
            for k in range(MH // 4):
                nc.vector.tensor_add(
                    qs_t[:, k * NB : k * NB + nb],
                    qs_t[:, 2 * k * NB : 2 * k * NB + nb],
                    qs_t[:, (2 * k + 1) * NB : (2 * k + 1) * NB + nb],
                )
            nc.vector.tensor_add(
                qs_t[:, :nb], qs_t[:, :nb], qs_t[:, NB : NB + nb]
            )
            ps_v = ps_stat.tile([P, NB], f32, tag="stat")
            nc.tensor.matmul(
                ps_v[:, :nb], lhsT=ones128[:], rhs=qs_t[:, :nb],
                start=True, stop=True,
            )
            sv = rs_pool.tile([P, NB], f32, tag="sv")
            nc.scalar.activation(
                sv[:, :nb], ps_v[:, :nb], AF.Sqrt,
                scale=1.0 / HIDDEN, bias=eps_ap[:],
            )
            rsig = rs_pool.tile([P, NB], f32, tag="rsig")
            nc.vector.reciprocal_approx_fast(rsig[:, :nb], sv[:, :nb])

            import functools

            pending.append(functools.partial(emit_tail, segs, c0, nb, ph_list, rsig))

        for p in pending:
            p()

    nc.compile()
    return nc


def plan(node_types):
    """Host-side layout plan shared by all cores.

    Column layout per core: [t x FULL*P for each type] + [t x P remainder for
    each type].  Returns (blocks, R, regions, idx_by_type) where regions[t] =
    (full_off, full_len, rem_off, rem_len) describes where type t's columns
    live, and idx_by_type[t][c] the original row indices for core c.
    """
    node_types = np.asarray(node_types)
    counts = np.bincount(node_types, minlength=TYPES)
    idx_by_type = []
    order = np.argsort(node_types, kind="stable")
    starts = np.concatenate([[0], np.cumsum(counts)])
    tiles_per_type = []
    for tt in range(TYPES):
        per_core = -(-int(counts[tt]) // N_CORES)
        tiles = -(-per_core // P)  # ceil to 128-row tiles per core
        tiles_per_type.append(tiles)
        idx_t = order[starts[tt] : starts[tt + 1]]
        base, rem = divmod(int(counts[tt]), N_CORES)
        parts, o = [], 0
        for c in range(N_CORES):
            n = base + (1 if c < rem else 0)
            parts.append(idx_t[o : o + n])
            o += n
        idx_by_type.append(parts)

    # full regions: whole 512-column blocks; exact-width remainders are
    # packed into one final mixed block padded up to a 128-column multiple
    per_core_max = [
        max(len(p) for p in idx_by_type[tt]) for tt in range(TYPES)
    ]
    fulls = [(m // NB) * NB for m in per_core_max]
    rems = [per_core_max[tt] - fulls[tt] for tt in range(TYPES)]
    rem_total = sum(rems)
    rem_cols = -(-rem_total // P) * P  # pad to 128-multiple
    regions = []
    blocks = []
    full_off = 0
    rem_base = sum(fulls)
    rem_off = rem_base
    for tt in range(TYPES):
        regions.append((full_off, fulls[tt], rem_off, rems[tt]))
        for j in range(fulls[tt] // NB):
            blocks.append(([(tt, NB)], full_off + j * NB, NB))
        full_off += fulls[tt]
        rem_off += rems[tt]
    if rem_cols:
        segs = [(tt, rems[tt]) for tt in range(TYPES) if rems[tt] > 0]
        segs[-1] = (segs[-1][0], segs[-1][1] + rem_cols - rem_total)
        assert rem_cols <= NB, "remainder block exceeds one NB block"
        blocks.append((segs, rem_base, rem_cols))
    R = rem_base + rem_cols
    return blocks, R, regions, idx_by_type


def prep_inputs(node_latent, w1, b1, w2, b2, ln_gamma, ln_beta, head_w, head_b,
                regions, idx_by_type, R):
    """Build the 8 per-core input maps, packed to the device SBUF layouts."""
    import ml_dtypes

    bf16 = ml_dtypes.bfloat16

    def cast(a):
        return np.asarray(a, dtype=np.float32).astype(bf16)

    w1 = np.asarray(w1, np.float32)
    w2 = np.asarray(w2, np.float32)
    b1 = np.asarray(b1, np.float32)
    b2 = np.asarray(b2, np.float32)
    # mean-center w2/b2 over the output dim: stage-2 output becomes zero-mean
    # for every input, which LayerNorm's mean subtraction makes exact
    w2c = w2 - w2.mean(axis=1, keepdims=True)
    b2c = b2 - b2.mean()
    whp = np.asarray(ln_gamma, np.float32)[None, :, None] * np.asarray(
        head_w, np.float32
    )  # [T, H, OUT]
    c1 = (np.asarray(ln_beta, np.float32) @ np.asarray(head_w, np.float32)
          + np.asarray(head_b, np.float32))  # [T, OUT]

    # [P, MH//2, KL*Q1] quarters: w1p[p, q, k*Q1 + j] = w1[k*128+p, q*Q1 + j]
    w1p = cast(w1.reshape(KL, P, MH // 2, Q1).transpose(1, 2, 0, 3)
               .reshape(P, MH // 2, KL * Q1))
    w2p = cast(w2c.reshape(KH, P, MH // 2, Q1).transpose(1, 2, 0, 3)
               .reshape(P, MH // 2, KH * Q1))
    whpp = cast(
        whp.reshape(TYPES, KH, P, OUT).transpose(2, 0, 1, 3).reshape(P, TYPES, KH * OUT)
    )
    b1r = np.ascontiguousarray(b1.reshape(MH, P).T).astype(np.float32)
    b2r = np.ascontiguousarray(b2c.reshape(MH, P).T).astype(np.float32)
    c1r = np.ascontiguousarray(
        c1.reshape(TYPES, MO, P).transpose(2, 0, 1).reshape(P, TYPES * MO)
    ).astype(np.float32)
    use_c1 = bool(np.any(c1))

    in_maps = []
    for c in range(N_CORES):
        xc = np.zeros((R, LATENT), np.float32)
        for tt in range(TYPES):
            fo, fl, ro, rl = regions[tt]
            idx = idx_by_type[tt][c]
            nf = min(len(idx), fl)
            xc[fo : fo + nf] = node_latent[idx[:nf]]
            xc[ro : ro + len(idx) - nf] = node_latent[idx[nf:]]
        xtp = cast(xc.T.reshape(KL, P, R).transpose(1, 0, 2))
        m = {
            "xt": xtp,
            "w1": w1p,
            "w2": w2p,
            "whp": whpp,
            "b1r": b1r,
            "b2r": b2r,
        }
        if use_c1:
            m["c1r"] = c1r
        in_maps.append(m)
    return in_maps, use_c1


def unpack_outputs(results, regions, idx_by_type, n_rows):
    out = np.empty((n_rows, OUT), np.float32)
    for c in range(N_CORES):
        oc = results[c]["out"]  # [P, MO, R]
        R = oc.shape[-1]
        flat = oc.transpose(2, 1, 0).reshape(R, OUT)  # node, (mc*P + p)
        for tt in range(TYPES):
            fo, fl, ro, rl = regions[tt]
            idx = idx_by_type[tt][c]
            nf = min(len(idx), fl)
            out[idx[:nf]] = flat[fo : fo + nf]
            out[idx[nf:]] = flat[ro : ro + len(idx) - nf]
    return out


def kernel(node_latent, node_types, w1, b1, w2, b2, ln_gamma, ln_beta, head_w, head_b):
    from concourse.bass_utils import run_bass_kernel_spmd

    node_latent = np.asarray(node_latent, dtype=np.float32)
    node_types = np.asarray(node_types)
    blocks, R, regions, idx_by_type = plan(node_types)
    in_maps, use_c1 = prep_inputs(
        node_latent, w1, b1, w2, b2, ln_gamma, ln_beta, head_w, head_b,
        regions, idx_by_type, R,
    )
    nc = build_program(blocks, R, use_c1=use_c1)
    res = run_bass_kernel_spmd(nc, in_maps, core_ids=list(range(N_CORES)))
    return unpack_outputs(res.results, regions, idx_by_type, node_latent.shape[0])


# revision 11
# speedup vs baseline: 1.0025x; 1.0025x over previous
"""Trainium2 Bass kernel for nn_AdaptiveDecoder (shared MLP + hard-routed type heads).

Strategy:
  * Host: sort nodes by type; each core gets the same static column layout:
    [t0 x 4096 | t1 x 4096 | t2 x 4096 | t0_rem x 128 | t1_rem x 128 | t2_rem x 128]
    so the compiled SPMD program bakes the tile->head mapping in and the
    device does zero routing work.  The three remainder tiles form one final
    384-column block whose head stage switches weights per 128-column segment,
    so the pipeline drains on a single small block.
  * Device: activations stay transposed ([feature, nodes]) so the three matmul
    stages chain without transposes.
  * w2/b2 are mean-centered on the host (per input row, subtract the output-dim
    mean) so stage-2 output is exactly zero-mean: LayerNorm's mean path
    vanishes and variance is just sum(h^2)/H.  The variance column-sum uses an
    all-ones [128,128] lhsT so 1/sigma lands replicated on all partitions --
    no broadcast matmul needed.
  * Stage-1 relu runs on ACT (DVE would gate psum-buffer reuse); DVE keeps the
    squares/pairwise-add/reciprocal chain and the final rsig multiply.
  * All weights packed on the host to the device SBUF layout; first-use
    ordered quarter-sized DMAs keep the startup critical path short.
"""

import sys

sys.path.insert(0, "/opt/trn_rl_repo")

from contextlib import ExitStack

import numpy as np

N_CORES = 8
LATENT, HIDDEN, OUT, TYPES = 512, 1024, 256, 3
P = 128
NB = 512  # node columns per block (psum bank limit for f32)
KL = LATENT // P  # 4 k-tiles, stage 1
KH = HIDDEN // P  # 8 k-tiles, stage 2 / head
MH = HIDDEN // P  # 8 m-chunks of hidden
MO = OUT // P  # 2 m-chunks of head output
Q1 = 256  # w1/w2 quarter width (2 m-chunks)
LN_EPS = 1e-5
FULL = (NB // P) * MH  # 32 full 128-tiles per type region (4096 cols)


def build_program(blocks, R, use_c1=False):
    """blocks: list of (types, col_offset, n_cols) where types is a list of
    (type_idx, seg_cols) head segments covering n_cols."""
    import concourse.mybir as mybir
    import concourse.tile as tile
    from concourse import bacc

    dt = mybir.dt
    f32, bf16 = dt.float32, dt.bfloat16
    AF = mybir.ActivationFunctionType

    nc = bacc.Bacc("TRN2", target_bir_lowering=False, debug=False, num_devices=N_CORES)

    xtd = nc.dram_tensor("xt", [P, KL, R], bf16, kind="ExternalInput").ap()
    w1d = nc.dram_tensor("w1", [P, MH // 2, KL * Q1], bf16, kind="ExternalInput").ap()
    w2d = nc.dram_tensor("w2", [P, MH // 2, KH * Q1], bf16, kind="ExternalInput").ap()
    whpd = nc.dram_tensor("whp", [P, TYPES, KH * OUT], bf16, kind="ExternalInput").ap()
    b1d = nc.dram_tensor("b1r", [P, MH], f32, kind="ExternalInput").ap()
    b2d = nc.dram_tensor("b2r", [P, MH], f32, kind="ExternalInput").ap()
    if use_c1:
        c1d = nc.dram_tensor("c1r", [P, TYPES * MO], f32, kind="ExternalInput").ap()
    outd = nc.dram_tensor("out", [P, MO, R], bf16, kind="ExternalOutput").ap()

    with tile.TileContext(nc) as tc, ExitStack() as ctx:
        consts = ctx.enter_context(tc.tile_pool(name="consts", bufs=1))
        xt_pool = ctx.enter_context(tc.tile_pool(name="xt", bufs=4))
        h1_pool = ctx.enter_context(tc.tile_pool(name="h1", bufs=2))
        h2_pool = ctx.enter_context(tc.tile_pool(name="h2", bufs=2))
        sq_pool = ctx.enter_context(tc.tile_pool(name="sq", bufs=1))
        qs_pool = ctx.enter_context(tc.tile_pool(name="qs", bufs=2))
        rs_pool = ctx.enter_context(tc.tile_pool(name="rs", bufs=2))
        out_pool = ctx.enter_context(tc.tile_pool(name="outp", bufs=2))
        ps_mlp = ctx.enter_context(tc.tile_pool(name="ps_mlp", bufs=4, space="PSUM"))
        ps_head = ctx.enter_context(tc.tile_pool(name="ps_head", bufs=2, space="PSUM"))
        ps_stat = ctx.enter_context(tc.tile_pool(name="ps_stat", bufs=2, space="PSUM"))

        # steady-state DMAs round-robin sync/gpsimd (ACT stays compute-only)
        dma_engines = [nc.sync, nc.gpsimd]
        dma_rr = [0]

        def dma(out, in_):
            eng = dma_engines[dma_rr[0] % len(dma_engines)]
            dma_rr[0] += 1
            eng.dma_start(out=out, in_=in_)

        xt_tiles = {}

        def load_xt(bi, eng=None, split=False):
            _, c0, nb = blocks[bi]
            xt_t = xt_pool.tile([P, KL, NB], bf16, tag="xt")
            if split:  # two k-halves so the first stage-1 group starts sooner
                for ks in range(2):
                    eng.dma_start(
                        out=xt_t[:, 2 * ks : 2 * ks + 2, :nb],
                        in_=xtd[:, 2 * ks : 2 * ks + 2, c0 : c0 + nb],
                    )
            elif eng is not None:
                eng.dma_start(out=xt_t[:, :, :nb], in_=xtd[:, :, c0 : c0 + nb])
            else:
                dma(xt_t[:, :, :nb], xtd[:, :, c0 : c0 + nb])
            xt_tiles[bi] = xt_t

        # --- prologue: interleave the critical-path weight quarters across
        # all four DMA queues in first-use (deadline) order ---
        w1_sb = consts.tile([P, MH // 2, KL * Q1], bf16)
        w2_sb = consts.tile([P, MH // 2, KH * Q1], bf16)
        b1_sb = consts.tile([P, MH], f32)
        b2_sb = consts.tile([P, MH], f32)
        whp_sb = consts.tile([P, TYPES, KH * OUT], bf16)
        t0_first = blocks[0][0][0][0]
        type_order = [t0_first] + [t for t in range(TYPES) if t != t0_first]

        _, _c0, _nb = blocks[0]
        xt0 = xt_pool.tile([P, KL, NB], bf16, tag="xt")
        xt_tiles[0] = xt0
        nc.sync.dma_start(out=xt0[:, 0:2, :_nb], in_=xtd[:, 0:2, _c0 : _c0 + _nb])
        nc.scalar.dma_start(out=w1_sb[:, 0, :], in_=w1d[:, 0, :])
        nc.gpsimd.dma_start(out=b1_sb[:], in_=b1d[:])
        nc.gpsimd.dma_start(out=xt0[:, 2:4, :_nb], in_=xtd[:, 2:4, _c0 : _c0 + _nb])
        nc.scalar.dma_start(out=w1_sb[:, 1, :], in_=w1d[:, 1, :])
        nc.sync.dma_start(out=w1_sb[:, 2, :], in_=w1d[:, 2, :])
        nc.gpsimd.dma_start(out=b2_sb[:], in_=b2d[:])
        nc.gpsimd.dma_start(out=w1_sb[:, 3, :], in_=w1d[:, 3, :])
        nc.scalar.dma_start(out=w2_sb[:, 0, :], in_=w2d[:, 0, :])
        nc.sync.dma_start(out=w2_sb[:, 1, :], in_=w2d[:, 1, :])
        nc.gpsimd.dma_start(out=w2_sb[:, 2, :], in_=w2d[:, 2, :])
        nc.gpsimd.dma_start(out=w2_sb[:, 3, :], in_=w2d[:, 3, :])
        nc.scalar.dma_start(
            out=whp_sb[:, type_order[0], :], in_=whpd[:, type_order[0], :]
        )
        for bi in range(1, min(3, len(blocks))):
            load_xt(bi, eng=(nc.sync if bi % 2 else nc.gpsimd))
        for ei, t in enumerate(type_order[1:]):
            (nc.scalar if ei % 2 else nc.gpsimd).dma_start(
                out=whp_sb[:, t, :], in_=whpd[:, t, :]
            )
        if use_c1:
            c1_sb = consts.tile([P, TYPES * MO], f32)
            nc.sync.dma_start(out=c1_sb[:], in_=c1d[:])

        ones128 = consts.tile([P, P], bf16)
        nc.vector.memset(ones128[:], 1.0)
        eps_ap = consts.tile([P, 1], f32)
        nc.vector.memset(eps_ap[:], LN_EPS)
        act_warm = consts.tile([1, 1], f32)
        nc.scalar.activation(act_warm[:], eps_ap[:1, :], AF.Sqrt)

        # --- per-block pipeline; tail (rsig multiply + output DMA) of block
        # b-1 is emitted at the top of block b so its DVE ops never gate the
        # PE and the output DMA issues as early as possible ---

        def emit_tail(segs, c0, nb, ph_list, rsig, last=False):
            out_sb = out_pool.tile([P, MO, NB], bf16, tag="out")
            for mc in range(MO):
                nc.vector.tensor_mul(
                    out_sb[:, mc, :nb], ph_list[mc][:, :nb], rsig[:, :nb]
                )
                if use_c1:
                    s0 = 0
                    for t, sw in segs:
                        nc.vector.tensor_scalar(
                            out_sb[:, mc, s0 : s0 + sw],
                            out_sb[:, mc, s0 : s0 + sw],
                            c1_sb[:, t * MO + mc : t * MO + mc + 1],
                            0.0,
                            op0=mybir.AluOpType.add,
                            op1=mybir.AluOpType.bypass,
                        )
                        s0 += sw
                if last:
                    (nc.sync if mc == 0 else nc.scalar).dma_start(
                        out=outd[:, mc, c0 : c0 + nb], in_=out_sb[:, mc, :nb]
                    )
                else:
                    dma(outd[:, mc, c0 : c0 + nb], out_sb[:, mc, :nb])

        pending = []
        for bi, (segs, c0, nb) in enumerate(blocks):
            xt_t = xt_tiles.pop(bi)
            if bi + 3 < len(blocks):
                load_xt(bi + 3)

            if pending:
                pending.pop(0)()

            # stage 1: h1^T = relu(W1^T x + b1)   [HIDDEN, nb]  (relu on ACT)
            h1_t = h1_pool.tile([P, MH * NB], bf16, tag="h1")
            for m in range(MH):
                q, i = divmod(m, 2)
                ps = ps_mlp.tile([P, NB], f32, tag="ps_mlp")
                for k in range(KL):
                    nc.tensor.matmul(
                        ps[:, :nb],
                        lhsT=w1_sb[:, q, k * Q1 + i * P : k * Q1 + (i + 1) * P],
                        rhs=xt_t[:, k, :nb],
                        start=(k == 0),
                        stop=(k == KL - 1),
                    )
                nc.scalar.activation(
                    h1_t[:, m * NB : m * NB + nb],
                    ps[:, :nb],
                    AF.Relu,
                    bias=b1_sb[:, m : m + 1],
                )

            # stage 2: h2^T = W2^T h1 + b2 (zero-mean by construction);
            # squares ride along per chunk for the variance sum
            h2_t = h2_pool.tile([P, MH * NB], bf16, tag="h2")
            sq_t = sq_pool.tile([P, MH * NB], bf16, tag="sq")
            qs_t = qs_pool.tile([P, (MH // 2) * NB], bf16, tag="qs")
            for m in range(MH):
                q, i = divmod(m, 2)
                ps = ps_mlp.tile([P, NB], f32, tag="ps_mlp")
                for k in range(KH):
                    nc.tensor.matmul(
                        ps[:, :nb],
                        lhsT=w2_sb[:, q, k * Q1 + i * P : k * Q1 + (i + 1) * P],
                        rhs=h1_t[:, k * NB : k * NB + nb],
                        start=(k == 0),
                        stop=(k == KH - 1),
                    )
                nc.scalar.activation(
                    h2_t[:, m * NB : m * NB + nb],
                    ps[:, :nb],
                    AF.Identity,
                    bias=b2_sb[:, m : m + 1],
                )
                nc.vector.tensor_mul(
                    sq_t[:, m * NB : m * NB + nb],
                    h2_t[:, m * NB : m * NB + nb],
                    h2_t[:, m * NB : m * NB + nb],
                )
                if m % 2 == 1:  # level-1 pairwise add as soon as a pair exists
                    k = m // 2
                    nc.vector.tensor_add(
                        qs_t[:, k * NB : k * NB + nb],
                        sq_t[:, 2 * k * NB : 2 * k * NB + nb],
                        sq_t[:, (2 * k + 1) * NB : (2 * k + 1) * NB + nb],
                    )

            # head main matmuls: keep the PE hot while the stats chain runs.
            # Mixed blocks switch head weights per 128-column segment.
            ph_list = []
            for mc in range(MO):
                ph = ps_head.tile([P, NB], f32, tag="head")
                s0 = 0
                for t, sw in segs:
                    for k in range(KH):
                        nc.tensor.matmul(
                            ph[:, s0 : s0 + sw],
                            lhsT=whp_sb[
                                :, t, k * OUT + mc * P : k * OUT + (mc + 1) * P
                            ],
                            rhs=h2_t[:, k * NB + s0 : k * NB + s0 + sw],
                            start=(k == 0),
                            stop=(k == KH - 1),
                        )
                    s0 += sw
                ph_list.append(ph)

            # variance: pairwise-add squares 8->4->2->1 on DVE, column-sum via
            # ones-matmul (result replicated on all 128 partitions), then
            # sigma = sqrt(sum/H + eps) on ACT and 1/sigma on DVE
            for k in range(MH // 4):
                nc.vector.tensor_add(
                    qs_t[:, k * NB : k * NB + nb],
                    qs_t[:, 2 * k * NB : 2 * k * NB + nb],
                    qs_t[:, (2 * k + 1) * NB : (2 * k + 1) * NB + nb],
                )
            nc.vector.tensor_add(
                qs_t[:, :nb], qs_t[:, :nb], qs_t[:, NB : NB + nb]
            )
            ps_v = ps_stat.tile([P, NB], f32, tag="stat")
            nc.tensor.matmul(
                ps_v[:, :nb], lhsT=ones128[:], rhs=qs_t[:, :nb],
                start=True, stop=True,
            )
            sv = rs_pool.tile([P, NB], f32, tag="sv")
            nc.scalar.activation(
                sv[:, :nb], ps_v[:, :nb], AF.Sqrt,
                scale=1.0 / HIDDEN, bias=eps_ap[:],
            )
            rsig = rs_pool.tile([P, NB], f32, tag="rsig")
            nc.vector.reciprocal_approx_fast(rsig[:, :nb], sv[:, :nb])

            import functools

            pending.append(functools.partial(
                emit_tail, segs, c0, nb, ph_list, rsig,
                last=(bi == len(blocks) - 1),
            ))

        for p in pending:
            p()

    nc.compile()
    return nc


def plan(node_types):
    """Host-side layout plan shared by all cores.

    Column layout per core: [t x FULL*P for each type] + [t x P remainder for
    each type].  Returns (blocks, R, regions, idx_by_type) where regions[t] =
    (full_off, full_len, rem_off, rem_len) describes where type t's columns
    live, and idx_by_type[t][c] the original row indices for core c.
    """
    node_types = np.asarray(node_types)
    counts = np.bincount(node_types, minlength=TYPES)
    idx_by_type = []
    order = np.argsort(node_types, kind="stable")
    starts = np.concatenate([[0], np.cumsum(counts)])
    tiles_per_type = []
    for tt in range(TYPES):
        per_core = -(-int(counts[tt]) // N_CORES)
        tiles = -(-per_core // P)  # ceil to 128-row tiles per core
        tiles_per_type.append(tiles)
        idx_t = order[starts[tt] : starts[tt + 1]]
        base, rem = divmod(int(counts[tt]), N_CORES)
        parts, o = [], 0
        for c in range(N_CORES):
            n = base + (1 if c < rem else 0)
            parts.append(idx_t[o : o + n])
            o += n
        idx_by_type.append(parts)

    # full regions: whole 512-column blocks; exact-width remainders are
    # packed into one final mixed block padded up to a 128-column multiple
    per_core_max = [
        max(len(p) for p in idx_by_type[tt]) for tt in range(TYPES)
    ]
    fulls = [(m // NB) * NB for m in per_core_max]
    rems = [per_core_max[tt] - fulls[tt] for tt in range(TYPES)]
    rem_total = sum(rems)
    rem_cols = -(-rem_total // P) * P  # pad to 128-multiple
    regions = []
    blocks = []
    full_off = 0
    rem_base = sum(fulls)
    rem_off = rem_base
    for tt in range(TYPES):
        regions.append((full_off, fulls[tt], rem_off, rems[tt]))
        for j in range(fulls[tt] // NB):
            blocks.append(([(tt, NB)], full_off + j * NB, NB))
        full_off += fulls[tt]
        rem_off += rems[tt]
    if rem_cols:
        segs = [(tt, rems[tt]) for tt in range(TYPES) if rems[tt] > 0]
        segs[-1] = (segs[-1][0], segs[-1][1] + rem_cols - rem_total)
        assert rem_cols <= NB, "remainder block exceeds one NB block"
        blocks.append((segs, rem_base, rem_cols))
    R = rem_base + rem_cols
    return blocks, R, regions, idx_by_type


def prep_inputs(node_latent, w1, b1, w2, b2, ln_gamma, ln_beta, head_w, head_b,
                regions, idx_by_type, R):
    """Build the 8 per-core input maps, packed to the device SBUF layouts."""
    import ml_dtypes

    bf16 = ml_dtypes.bfloat16

    def cast(a):
        return np.asarray(a, dtype=np.float32).astype(bf16)

    w1 = np.asarray(w1, np.float32)
    w2 = np.asarray(w2, np.float32)
    b1 = np.asarray(b1, np.float32)
    b2 = np.asarray(b2, np.float32)
    # mean-center w2/b2 over the output dim: stage-2 output becomes zero-mean
    # for every input, which LayerNorm's mean subtraction makes exact
    w2c = w2 - w2.mean(axis=1, keepdims=True)
    b2c = b2 - b2.mean()
    whp = np.asarray(ln_gamma, np.float32)[None, :, None] * np.asarray(
        head_w, np.float32
    )  # [T, H, OUT]
    c1 = (np.asarray(ln_beta, np.float32) @ np.asarray(head_w, np.float32)
          + np.asarray(head_b, np.float32))  # [T, OUT]

    # [P, MH//2, KL*Q1] quarters: w1p[p, q, k*Q1 + j] = w1[k*128+p, q*Q1 + j]
    w1p = cast(w1.reshape(KL, P, MH // 2, Q1).transpose(1, 2, 0, 3)
               .reshape(P, MH // 2, KL * Q1))
    w2p = cast(w2c.reshape(KH, P, MH // 2, Q1).transpose(1, 2, 0, 3)
               .reshape(P, MH // 2, KH * Q1))
    whpp = cast(
        whp.reshape(TYPES, KH, P, OUT).transpose(2, 0, 1, 3).reshape(P, TYPES, KH * OUT)
    )
    b1r = np.ascontiguousarray(b1.reshape(MH, P).T).astype(np.float32)
    b2r = np.ascontiguousarray(b2c.reshape(MH, P).T).astype(np.float32)
    c1r = np.ascontiguousarray(
        c1.reshape(TYPES, MO, P).transpose(2, 0, 1).reshape(P, TYPES * MO)
    ).astype(np.float32)
    use_c1 = bool(np.any(c1))

    in_maps = []
    for c in range(N_CORES):
        xc = np.zeros((R, LATENT), np.float32)
        for tt in range(TYPES):
            fo, fl, ro, rl = regions[tt]
            idx = idx_by_type[tt][c]
            nf = min(len(idx), fl)
            xc[fo : fo + nf] = node_latent[idx[:nf]]
            xc[ro : ro + len(idx) - nf] = node_latent[idx[nf:]]
        xtp = cast(xc.T.reshape(KL, P, R).transpose(1, 0, 2))
        m = {
            "xt": xtp,
            "w1": w1p,
            "w2": w2p,
            "whp": whpp,
            "b1r": b1r,
            "b2r": b2r,
        }
        if use_c1:
            m["c1r"] = c1r
        in_maps.append(m)
    return in_maps, use_c1


def unpack_outputs(results, regions, idx_by_type, n_rows):
    out = np.empty((n_rows, OUT), np.float32)
    for c in range(N_CORES):
        oc = results[c]["out"]  # [P, MO, R]
        R = oc.shape[-1]
        flat = oc.transpose(2, 1, 0).reshape(R, OUT)  # node, (mc*P + p)
        for tt in range(TYPES):
            fo, fl, ro, rl = regions[tt]
            idx = idx_by_type[tt][c]
            nf = min(len(idx), fl)
            out[idx[:nf]] = flat[fo : fo + nf]
            out[idx[nf:]] = flat[ro : ro + len(idx) - nf]
    return out


def kernel(node_latent, node_types, w1, b1, w2, b2, ln_gamma, ln_beta, head_w, head_b):
    from concourse.bass_utils import run_bass_kernel_spmd

    node_latent = np.asarray(node_latent, dtype=np.float32)
    node_types = np.asarray(node_types)
    blocks, R, regions, idx_by_type = plan(node_types)
    in_maps, use_c1 = prep_inputs(
        node_latent, w1, b1, w2, b2, ln_gamma, ln_beta, head_w, head_b,
        regions, idx_by_type, R,
    )
    nc = build_program(blocks, R, use_c1=use_c1)
    res = run_bass_kernel_spmd(nc, in_maps, core_ids=list(range(N_CORES)))
    return unpack_outputs(res.results, regions, idx_by_type, node_latent.shape[0])


# revision 12
# speedup vs baseline: 1.0081x; 1.0056x over previous
"""Trainium2 Bass kernel for nn_AdaptiveDecoder (shared MLP + hard-routed type heads).

Strategy:
  * Host: sort nodes by type; each core gets the same static column layout:
    [t0 x 4096 | t1 x 4096 | t2 x 4096 | t0_rem x 128 | t1_rem x 128 | t2_rem x 128]
    so the compiled SPMD program bakes the tile->head mapping in and the
    device does zero routing work.  The three remainder tiles form one final
    384-column block whose head stage switches weights per 128-column segment,
    so the pipeline drains on a single small block.
  * Device: activations stay transposed ([feature, nodes]) so the three matmul
    stages chain without transposes.
  * w2/b2 are mean-centered on the host (per input row, subtract the output-dim
    mean) so stage-2 output is exactly zero-mean: LayerNorm's mean path
    vanishes and variance is just sum(h^2)/H.  The variance column-sum uses an
    all-ones [128,128] lhsT so 1/sigma lands replicated on all partitions --
    no broadcast matmul needed.
  * Stage-1 relu runs on ACT (DVE would gate psum-buffer reuse); DVE keeps the
    squares/pairwise-add/reciprocal chain and the final rsig multiply.
  * All weights packed on the host to the device SBUF layout; first-use
    ordered quarter-sized DMAs keep the startup critical path short.
"""

import sys

sys.path.insert(0, "/opt/trn_rl_repo")

from contextlib import ExitStack

import numpy as np

N_CORES = 8
LATENT, HIDDEN, OUT, TYPES = 512, 1024, 256, 3
P = 128
NB = 512  # node columns per block (psum bank limit for f32)
KL = LATENT // P  # 4 k-tiles, stage 1
KH = HIDDEN // P  # 8 k-tiles, stage 2 / head
MH = HIDDEN // P  # 8 m-chunks of hidden
MO = OUT // P  # 2 m-chunks of head output
Q1 = 256  # w1/w2 quarter width (2 m-chunks)
LN_EPS = 1e-5
FULL = (NB // P) * MH  # 32 full 128-tiles per type region (4096 cols)


def build_program(blocks, R, use_c1=False):
    """blocks: list of (types, col_offset, n_cols) where types is a list of
    (type_idx, seg_cols) head segments covering n_cols."""
    import concourse.mybir as mybir
    import concourse.tile as tile
    from concourse import bacc

    dt = mybir.dt
    f32, bf16 = dt.float32, dt.bfloat16
    AF = mybir.ActivationFunctionType

    nc = bacc.Bacc("TRN2", target_bir_lowering=False, debug=False, num_devices=N_CORES)

    xtd = nc.dram_tensor("xt", [P, KL, R], bf16, kind="ExternalInput").ap()
    w1d = nc.dram_tensor("w1", [P, MH // 2, KL * Q1], bf16, kind="ExternalInput").ap()
    w2d = nc.dram_tensor("w2", [P, MH // 2, KH * Q1], bf16, kind="ExternalInput").ap()
    whpd = nc.dram_tensor("whp", [P, TYPES, KH * OUT], bf16, kind="ExternalInput").ap()
    b1d = nc.dram_tensor("b1r", [P, MH], f32, kind="ExternalInput").ap()
    b2d = nc.dram_tensor("b2r", [P, MH], f32, kind="ExternalInput").ap()
    if use_c1:
        c1d = nc.dram_tensor("c1r", [P, TYPES * MO], f32, kind="ExternalInput").ap()
    outd = nc.dram_tensor("out", [P, MO, R], bf16, kind="ExternalOutput").ap()

    with tile.TileContext(nc) as tc, ExitStack() as ctx:
        consts = ctx.enter_context(tc.tile_pool(name="consts", bufs=1))
        xt_pool = ctx.enter_context(tc.tile_pool(name="xt", bufs=4))
        h1_pool = ctx.enter_context(tc.tile_pool(name="h1", bufs=2))
        h2_pool = ctx.enter_context(tc.tile_pool(name="h2", bufs=2))
        sq_pool = ctx.enter_context(tc.tile_pool(name="sq", bufs=1))
        qs_pool = ctx.enter_context(tc.tile_pool(name="qs", bufs=2))
        rs_pool = ctx.enter_context(tc.tile_pool(name="rs", bufs=2))
        out_pool = ctx.enter_context(tc.tile_pool(name="outp", bufs=2))
        ps_mlp = ctx.enter_context(tc.tile_pool(name="ps_mlp", bufs=4, space="PSUM"))
        ps_head = ctx.enter_context(tc.tile_pool(name="ps_head", bufs=2, space="PSUM"))
        ps_stat = ctx.enter_context(tc.tile_pool(name="ps_stat", bufs=2, space="PSUM"))

        # steady-state DMAs round-robin sync/gpsimd (ACT stays compute-only)
        dma_engines = [nc.sync, nc.gpsimd]
        dma_rr = [0]

        def dma(out, in_):
            eng = dma_engines[dma_rr[0] % len(dma_engines)]
            dma_rr[0] += 1
            eng.dma_start(out=out, in_=in_)

        xt_tiles = {}

        def load_xt(bi, eng=None, split=False):
            _, c0, nb = blocks[bi]
            xt_t = xt_pool.tile([P, KL, NB], bf16, tag="xt")
            if split:  # two k-halves so the first stage-1 group starts sooner
                for ks in range(2):
                    eng.dma_start(
                        out=xt_t[:, 2 * ks : 2 * ks + 2, :nb],
                        in_=xtd[:, 2 * ks : 2 * ks + 2, c0 : c0 + nb],
                    )
            elif eng is not None:
                eng.dma_start(out=xt_t[:, :, :nb], in_=xtd[:, :, c0 : c0 + nb])
            else:
                dma(xt_t[:, :, :nb], xtd[:, :, c0 : c0 + nb])
            xt_tiles[bi] = xt_t

        # --- prologue: interleave the critical-path weight quarters across
        # all four DMA queues in first-use (deadline) order ---
        w1_sb = consts.tile([P, MH // 2, KL * Q1], bf16)
        w2_sb = consts.tile([P, MH // 2, KH * Q1], bf16)
        b1_sb = consts.tile([P, MH], f32)
        b2_sb = consts.tile([P, MH], f32)
        whp_sb = consts.tile([P, TYPES, KH * OUT], bf16)
        t0_first = blocks[0][0][0][0]
        type_order = [t0_first] + [t for t in range(TYPES) if t != t0_first]

        _, _c0, _nb = blocks[0]
        xt0 = xt_pool.tile([P, KL, NB], bf16, tag="xt")
        xt_tiles[0] = xt0
        nc.sync.dma_start(out=xt0[:, 0:2, :_nb], in_=xtd[:, 0:2, _c0 : _c0 + _nb])
        nc.scalar.dma_start(out=w1_sb[:, 0, :], in_=w1d[:, 0, :])
        nc.gpsimd.dma_start(out=b1_sb[:], in_=b1d[:])
        nc.gpsimd.dma_start(out=xt0[:, 2:4, :_nb], in_=xtd[:, 2:4, _c0 : _c0 + _nb])
        nc.scalar.dma_start(out=w1_sb[:, 1, :], in_=w1d[:, 1, :])
        nc.sync.dma_start(out=w1_sb[:, 2, :], in_=w1d[:, 2, :])
        nc.gpsimd.dma_start(out=b2_sb[:], in_=b2d[:])
        nc.gpsimd.dma_start(out=w1_sb[:, 3, :], in_=w1d[:, 3, :])
        nc.scalar.dma_start(out=w2_sb[:, 0, :], in_=w2d[:, 0, :])
        nc.sync.dma_start(out=w2_sb[:, 1, :], in_=w2d[:, 1, :])
        nc.gpsimd.dma_start(out=w2_sb[:, 2, :], in_=w2d[:, 2, :])
        nc.gpsimd.dma_start(out=w2_sb[:, 3, :], in_=w2d[:, 3, :])
        nc.scalar.dma_start(
            out=whp_sb[:, type_order[0], :], in_=whpd[:, type_order[0], :]
        )
        for bi in range(1, min(3, len(blocks))):
            load_xt(bi, eng=(nc.sync if bi % 2 else nc.gpsimd))
        for ei, t in enumerate(type_order[1:]):
            (nc.scalar if ei % 2 else nc.gpsimd).dma_start(
                out=whp_sb[:, t, :], in_=whpd[:, t, :]
            )
        if use_c1:
            c1_sb = consts.tile([P, TYPES * MO], f32)
            nc.sync.dma_start(out=c1_sb[:], in_=c1d[:])

        ones128 = consts.tile([P, P], bf16)
        nc.vector.memset(ones128[:], 1.0)
        eps_ap = consts.tile([P, 1], f32)
        nc.vector.memset(eps_ap[:], LN_EPS)
        act_warm = consts.tile([1, 1], f32)
        nc.scalar.activation(act_warm[:], eps_ap[:1, :], AF.Sqrt)
        # dummy matmuls ramp the PE p-state to full clock while the first
        # input/weight DMAs are still in flight
        warm_rhs = consts.tile([P, NB], bf16)
        nc.vector.memset(warm_rhs[:], 0.0)
        for _ in range(20):
            ps_w = ps_stat.tile([P, NB], f32, tag="stat")
            nc.tensor.matmul(
                ps_w[:], lhsT=ones128[:], rhs=warm_rhs[:], start=True, stop=True
            )

        # --- per-block pipeline; tail (rsig multiply + output DMA) of block
        # b-1 is emitted at the top of block b so its DVE ops never gate the
        # PE and the output DMA issues as early as possible ---

        def emit_tail(segs, c0, nb, ph_list, rsig, last=False):
            out_sb = out_pool.tile([P, MO, NB], bf16, tag="out")
            for mc in range(MO):
                nc.vector.tensor_mul(
                    out_sb[:, mc, :nb], ph_list[mc][:, :nb], rsig[:, :nb]
                )
                if use_c1:
                    s0 = 0
                    for t, sw in segs:
                        nc.vector.tensor_scalar(
                            out_sb[:, mc, s0 : s0 + sw],
                            out_sb[:, mc, s0 : s0 + sw],
                            c1_sb[:, t * MO + mc : t * MO + mc + 1],
                            0.0,
                            op0=mybir.AluOpType.add,
                            op1=mybir.AluOpType.bypass,
                        )
                        s0 += sw
                if last:
                    (nc.sync if mc == 0 else nc.scalar).dma_start(
                        out=outd[:, mc, c0 : c0 + nb], in_=out_sb[:, mc, :nb]
                    )
                else:
                    dma(outd[:, mc, c0 : c0 + nb], out_sb[:, mc, :nb])

        pending = []
        for bi, (segs, c0, nb) in enumerate(blocks):
            xt_t = xt_tiles.pop(bi)
            if bi + 3 < len(blocks):
                load_xt(bi + 3)

            if pending:
                pending.pop(0)()

            # stage 1: h1^T = relu(W1^T x + b1)   [HIDDEN, nb]  (relu on ACT)
            h1_t = h1_pool.tile([P, MH * NB], bf16, tag="h1")
            for m in range(MH):
                q, i = divmod(m, 2)
                ps = ps_mlp.tile([P, NB], f32, tag="ps_mlp")
                for k in range(KL):
                    nc.tensor.matmul(
                        ps[:, :nb],
                        lhsT=w1_sb[:, q, k * Q1 + i * P : k * Q1 + (i + 1) * P],
                        rhs=xt_t[:, k, :nb],
                        start=(k == 0),
                        stop=(k == KL - 1),
                    )
                nc.scalar.activation(
                    h1_t[:, m * NB : m * NB + nb],
                    ps[:, :nb],
                    AF.Relu,
                    bias=b1_sb[:, m : m + 1],
                )

            # stage 2: h2^T = W2^T h1 + b2 (zero-mean by construction);
            # squares ride along per chunk for the variance sum
            h2_t = h2_pool.tile([P, MH * NB], bf16, tag="h2")
            sq_t = sq_pool.tile([P, MH * NB], bf16, tag="sq")
            qs_t = qs_pool.tile([P, (MH // 2) * NB], bf16, tag="qs")
            for m in range(MH):
                q, i = divmod(m, 2)
                ps = ps_mlp.tile([P, NB], f32, tag="ps_mlp")
                for k in range(KH):
                    nc.tensor.matmul(
                        ps[:, :nb],
                        lhsT=w2_sb[:, q, k * Q1 + i * P : k * Q1 + (i + 1) * P],
                        rhs=h1_t[:, k * NB : k * NB + nb],
                        start=(k == 0),
                        stop=(k == KH - 1),
                    )
                nc.scalar.activation(
                    h2_t[:, m * NB : m * NB + nb],
                    ps[:, :nb],
                    AF.Identity,
                    bias=b2_sb[:, m : m + 1],
                )
                nc.vector.tensor_mul(
                    sq_t[:, m * NB : m * NB + nb],
                    h2_t[:, m * NB : m * NB + nb],
                    h2_t[:, m * NB : m * NB + nb],
                )
                if m % 2 == 1:  # pairwise-add tree as soon as pairs exist
                    k = m // 2
                    nc.vector.tensor_add(
                        qs_t[:, k * NB : k * NB + nb],
                        sq_t[:, 2 * k * NB : 2 * k * NB + nb],
                        sq_t[:, (2 * k + 1) * NB : (2 * k + 1) * NB + nb],
                    )
                if m % 4 == 3:  # level-2
                    k = m // 4
                    nc.vector.tensor_add(
                        qs_t[:, k * NB : k * NB + nb],
                        qs_t[:, 2 * k * NB : 2 * k * NB + nb],
                        qs_t[:, (2 * k + 1) * NB : (2 * k + 1) * NB + nb],
                    )

            # head main matmuls: keep the PE hot while the stats chain runs.
            # Mixed blocks switch head weights per 128-column segment.
            ph_list = []
            for mc in range(MO):
                ph = ps_head.tile([P, NB], f32, tag="head")
                s0 = 0
                for t, sw in segs:
                    for k in range(KH):
                        nc.tensor.matmul(
                            ph[:, s0 : s0 + sw],
                            lhsT=whp_sb[
                                :, t, k * OUT + mc * P : k * OUT + (mc + 1) * P
                            ],
                            rhs=h2_t[:, k * NB + s0 : k * NB + s0 + sw],
                            start=(k == 0),
                            stop=(k == KH - 1),
                        )
                    s0 += sw
                ph_list.append(ph)

            # variance: pairwise-add squares 8->4->2->1 on DVE, column-sum via
            # ones-matmul (result replicated on all 128 partitions), then
            # sigma = sqrt(sum/H + eps) on ACT and 1/sigma on DVE
            nc.vector.tensor_add(
                qs_t[:, :nb], qs_t[:, :nb], qs_t[:, NB : NB + nb]
            )
            ps_v = ps_stat.tile([P, NB], f32, tag="stat")
            nc.tensor.matmul(
                ps_v[:, :nb], lhsT=ones128[:], rhs=qs_t[:, :nb],
                start=True, stop=True,
            )
            sv = rs_pool.tile([P, NB], f32, tag="sv")
            nc.scalar.activation(
                sv[:, :nb], ps_v[:, :nb], AF.Sqrt,
                scale=1.0 / HIDDEN, bias=eps_ap[:],
            )
            rsig = rs_pool.tile([P, NB], f32, tag="rsig")
            nc.vector.reciprocal_approx_fast(rsig[:, :nb], sv[:, :nb])

            import functools

            pending.append(functools.partial(
                emit_tail, segs, c0, nb, ph_list, rsig,
                last=(bi == len(blocks) - 1),
            ))

        for p in pending:
            p()

    nc.compile()
    return nc


def plan(node_types):
    """Host-side layout plan shared by all cores.

    Column layout per core: [t x FULL*P for each type] + [t x P remainder for
    each type].  Returns (blocks, R, regions, idx_by_type) where regions[t] =
    (full_off, full_len, rem_off, rem_len) describes where type t's columns
    live, and idx_by_type[t][c] the original row indices for core c.
    """
    node_types = np.asarray(node_types)
    counts = np.bincount(node_types, minlength=TYPES)
    idx_by_type = []
    order = np.argsort(node_types, kind="stable")
    starts = np.concatenate([[0], np.cumsum(counts)])
    tiles_per_type = []
    for tt in range(TYPES):
        per_core = -(-int(counts[tt]) // N_CORES)
        tiles = -(-per_core // P)  # ceil to 128-row tiles per core
        tiles_per_type.append(tiles)
        idx_t = order[starts[tt] : starts[tt + 1]]
        base, rem = divmod(int(counts[tt]), N_CORES)
        parts, o = [], 0
        for c in range(N_CORES):
            n = base + (1 if c < rem else 0)
            parts.append(idx_t[o : o + n])
            o += n
        idx_by_type.append(parts)

    # full regions: whole 512-column blocks; exact-width remainders are
    # packed into one final mixed block padded up to a 128-column multiple
    per_core_max = [
        max(len(p) for p in idx_by_type[tt]) for tt in range(TYPES)
    ]
    fulls = [(m // NB) * NB for m in per_core_max]
    rems = [per_core_max[tt] - fulls[tt] for tt in range(TYPES)]
    rem_total = sum(rems)
    rem_cols = -(-rem_total // P) * P  # pad to 128-multiple
    regions = []
    blocks = []
    full_off = 0
    rem_base = sum(fulls)
    rem_off = rem_base
    for tt in range(TYPES):
        regions.append((full_off, fulls[tt], rem_off, rems[tt]))
        for j in range(fulls[tt] // NB):
            blocks.append(([(tt, NB)], full_off + j * NB, NB))
        full_off += fulls[tt]
        rem_off += rems[tt]
    if rem_cols:
        segs = [(tt, rems[tt]) for tt in range(TYPES) if rems[tt] > 0]
        segs[-1] = (segs[-1][0], segs[-1][1] + rem_cols - rem_total)
        assert rem_cols <= NB, "remainder block exceeds one NB block"
        blocks.append((segs, rem_base, rem_cols))
    R = rem_base + rem_cols
    return blocks, R, regions, idx_by_type


def prep_inputs(node_latent, w1, b1, w2, b2, ln_gamma, ln_beta, head_w, head_b,
                regions, idx_by_type, R):
    """Build the 8 per-core input maps, packed to the device SBUF layouts."""
    import ml_dtypes

    bf16 = ml_dtypes.bfloat16

    def cast(a):
        return np.asarray(a, dtype=np.float32).astype(bf16)

    w1 = np.asarray(w1, np.float32)
    w2 = np.asarray(w2, np.float32)
    b1 = np.asarray(b1, np.float32)
    b2 = np.asarray(b2, np.float32)
    # mean-center w2/b2 over the output dim: stage-2 output becomes zero-mean
    # for every input, which LayerNorm's mean subtraction makes exact
    w2c = w2 - w2.mean(axis=1, keepdims=True)
    b2c = b2 - b2.mean()
    whp = np.asarray(ln_gamma, np.float32)[None, :, None] * np.asarray(
        head_w, np.float32
    )  # [T, H, OUT]
    c1 = (np.asarray(ln_beta, np.float32) @ np.asarray(head_w, np.float32)
          + np.asarray(head_b, np.float32))  # [T, OUT]

    # [P, MH//2, KL*Q1] quarters: w1p[p, q, k*Q1 + j] = w1[k*128+p, q*Q1 + j]
    w1p = cast(w1.reshape(KL, P, MH // 2, Q1).transpose(1, 2, 0, 3)
               .reshape(P, MH // 2, KL * Q1))
    w2p = cast(w2c.reshape(KH, P, MH // 2, Q1).transpose(1, 2, 0, 3)
               .reshape(P, MH // 2, KH * Q1))
    whpp = cast(
        whp.reshape(TYPES, KH, P, OUT).transpose(2, 0, 1, 3).reshape(P, TYPES, KH * OUT)
    )
    b1r = np.ascontiguousarray(b1.reshape(MH, P).T).astype(np.float32)
    b2r = np.ascontiguousarray(b2c.reshape(MH, P).T).astype(np.float32)
    c1r = np.ascontiguousarray(
        c1.reshape(TYPES, MO, P).transpose(2, 0, 1).reshape(P, TYPES * MO)
    ).astype(np.float32)
    use_c1 = bool(np.any(c1))

    in_maps = []
    for c in range(N_CORES):
        xc = np.zeros((R, LATENT), np.float32)
        for tt in range(TYPES):
            fo, fl, ro, rl = regions[tt]
            idx = idx_by_type[tt][c]
            nf = min(len(idx), fl)
            xc[fo : fo + nf] = node_latent[idx[:nf]]
            xc[ro : ro + len(idx) - nf] = node_latent[idx[nf:]]
        xtp = cast(xc.T.reshape(KL, P, R).transpose(1, 0, 2))
        m = {
            "xt": xtp,
            "w1": w1p,
            "w2": w2p,
            "whp": whpp,
            "b1r": b1r,
            "b2r": b2r,
        }
        if use_c1:
            m["c1r"] = c1r
        in_maps.append(m)
    return in_maps, use_c1


def unpack_outputs(results, regions, idx_by_type, n_rows):
    out = np.empty((n_rows, OUT), np.float32)
    for c in range(N_CORES):
        oc = results[c]["out"]  # [P, MO, R]
        R = oc.shape[-1]
        flat = oc.transpose(2, 1, 0).reshape(R, OUT)  # node, (mc*P + p)
        for tt in range(TYPES):
            fo, fl, ro, rl = regions[tt]
            idx = idx_by_type[tt][c]
            nf = min(len(idx), fl)
            out[idx[:nf]] = flat[fo : fo + nf]
            out[idx[nf:]] = flat[ro : ro + len(idx) - nf]
    return out


def kernel(node_latent, node_types, w1, b1, w2, b2, ln_gamma, ln_beta, head_w, head_b):
    from concourse.bass_utils import run_bass_kernel_spmd

    node_latent = np.asarray(node_latent, dtype=np.float32)
    node_types = np.asarray(node_types)
    blocks, R, regions, idx_by_type = plan(node_types)
    in_maps, use_c1 = prep_inputs(
        node_latent, w1, b1, w2, b2, ln_gamma, ln_beta, head_w, head_b,
        regions, idx_by_type, R,
    )
    nc = build_program(blocks, R, use_c1=use_c1)
    res = run_bass_kernel_spmd(nc, in_maps, core_ids=list(range(N_CORES)))
    return unpack_outputs(res.results, regions, idx_by_type, node_latent.shape[0])
